# revision 32
# baseline (speedup 1.0000x reference)
"""Multi-head attention (B=2, S=2048, D=1024, H=16, dh=64) on 8 Trainium2 cores.

Sharding: head-tensor-parallel x batch. Core c owns batch b=c//4 and heads
4*(c%4)..4*(c%4)+3 (256 of the 1024 ctx dims). Each core computes its heads'
Q/K/V projections, attention, and a partial output projection against its
256 rows of Wo (+ bo/4 so the 4 partials per batch sum to one bo). The host
unshard step sums the 4 partial outputs per batch.

Per-core kernel (fp16 matmul operands, fp32 PSUM accumulation) — v3:
  Same math as the fp16 baseline: qT/kT = W.T@x.T computed in transposed
  form; head pairs row-packed so the two K=64 score matmuls of a pair run
  concurrently in PE row groups; exp on ScalarE ([128,1024] per key tile,
  ~1.1us — the ACT engine is the # 2 bottleneck at ~160us); A@V with a
  stationary [V | ones] operand so softmax denominators fall out of the
  accumulation; 1/rowsum via Ln+Exp(-x) on ACT; per-head broadcast via K=1
  fp16 matmul; out^T = Wo.T @ ctx_norm^T.

  v3 scheduling changes vs the 259.8us baseline (trace-driven):
  - Inputs are host-prepacked into SBUF-image layouts so each tensor is ONE
    contiguous DMA descriptor (12 total vs ~46), and issuance alternates the
    Sync/GpSimd queues: descriptor issue is ~0.6us each on one queue and was
    serializing the prologue (first exp at 29us; now ~13us).
  - Projection groups emit in 4-matmul halves so no more than ~0.9us of
    filler PE work ever sits between two score pairs (the 8-matmul groups
    caused 2.1us exp gaps at every slice boundary).
  - Output DMAs alternate Sync/GpSimd queues.
  - The last query slice runs as two 256-wide half-slices so the final
    normalize + out-projection tail drains in ~6us instead of ~13us.
"""

import numpy as np

import bass_rust
import concourse.bass as bass
import concourse.mybir as mybir
import concourse.tile as tile
from concourse.bass_utils import run_bass_kernel_spmd

B = 2
S = 2048
D = 1024
H = 16
DH = 64
OUT = 1024
NCORES = 8
HPC = H // 4  # heads per core = 4
E = HPC * DH  # 256 ctx dims per core
EV = HPC * (DH + 1)  # 260: v with interleaved ones columns

FP16 = mybir.dt.float16
FP32 = mybir.dt.float32

SCALE = 1.0 / float(np.sqrt(DH))

KT = D // 128  # 8 k-tiles for projections
MT = S // 128  # 16 key-token tiles
NQ = S // 512  # 4 query slices of 512


def _split_waits(nc, maxw=1):
    """This container's walrus rejects instructions carrying more than one
    semaphore wait ("Too many sync wait commands"); hoist extras onto
    standalone same-engine nops, preserving per-engine program order."""
    for bb in nc.main_func.blocks:
        new_il = []
        for inst in bb.instructions:
            si = inst.sync_info
            if si is not None and si.on_wait and len(si.on_wait) > maxw:
                waits = list(si.on_wait)
                for j, w in enumerate(waits[:-maxw]):
                    nop = mybir.InstNoOp(
                        name=f"{inst.name}-ws{j}", ins=[], outs=[], engine=inst.engine
                    )
                    nop.sync_info = bass_rust.SyncInfo(on_wait=[w], on_update=[])
                    new_il.append(nop)
                inst.sync_info = bass_rust.SyncInfo(
                    on_wait=waits[-maxw:], on_update=list(si.on_update)
                )
            new_il.append(inst)
        bb.instructions = new_il


def _prune_waits(nc):
    """Drop provably-satisfied semaphore waits before _split_waits:

    1. Same-engine waits: engines execute/dispatch in program order, and
       every sem-inc updater of an engine-local semaphore fires at
       instruction completion in that order. A wait on a semaphore whose
       only updaters are earlier instructions of the SAME engine, with
       threshold <= the number of those updates already emitted, is
       implied by program order. (DMA-queue sems are excluded: their
       increments fire at asynchronous DMA completion.)
    2. Dominated waits: semaphores are monotone counters, and an engine
       dispatches in order, so a wait with threshold <= the max threshold
       this engine has already waited for on the same semaphore is a
       no-op.
    Both classes otherwise cost sequencer wait-checks and, via
    _split_waits, standalone NOP dispatches on the busiest engines."""
    # sem id -> set of updater engines; and async (DMA) sems
    updaters = {}
    async_sems = set()
    for bb in nc.main_func.blocks:
        for inst in bb.instructions:
            si = inst.sync_info
            if si is None:
                continue
            for u in si.on_update:
                if getattr(u, "update_mode", None) != "sem-inc":
                    async_sems.add(u.id)
                    continue
                updaters.setdefault(u.id, set()).add(inst.engine)
                if type(inst).__name__ in ("InstDMACopy", "InstDMACopyTranspose"):
                    async_sems.add(u.id)

    emitted = {}  # (engine, sem id) -> inc count emitted so far
    waited = {}  # (engine, sem id) -> max threshold already waited
    pruned = [0, 0]
    for bb in nc.main_func.blocks:
        for inst in bb.instructions:
            si = inst.sync_info
            if si is None:
                continue
            eng = inst.engine
            if si.on_wait:
                keep = []
                for w in si.on_wait:
                    if getattr(w, "wait_mode", None) != "sem-ge-imm":
                        keep.append(w)
                        continue
                    sid, val = w.id, w.wait_value
                    if sid not in async_sems:
                        if updaters.get(sid) == {eng} and val <= emitted.get(
                            (eng, sid), 0
                        ):
                            pruned[0] += 1
                            continue
                        if val <= waited.get((eng, sid), -1):
                            pruned[1] += 1
                            continue
                    waited[(eng, sid)] = max(waited.get((eng, sid), -1), val)
                    keep.append(w)
                if len(keep) != len(si.on_wait):
                    inst.sync_info = bass_rust.SyncInfo(
                        on_wait=keep, on_update=list(si.on_update)
                    )
            for u in si.on_update:
                if getattr(u, "update_mode", None) == "sem-inc":
                    k = (eng, u.id)
                    emitted[k] = emitted.get(k, 0) + u.update_value


def build_program():
    nc = bass.Bass()

    # Host-prepacked SBUF images: one contiguous DMA descriptor per tensor.
    # xall[p, (n*8 + k)*512 + c] = x[n*512+c, k*128+p]
    xall_d = nc.declare_dram_parameter("xall", [128, 8 * S], FP16, isOutput=False)
    # wq/wk[p, k*256 + e] = W[k*128+p, e]
    wq_d = nc.declare_dram_parameter("wq", [128, KT * E], FP16, isOutput=False)
    wk_d = nc.declare_dram_parameter("wk", [128, KT * E], FP16, isOutput=False)
    # wv[p, k*256 + e] = Wv[k*128+p, e] (same image as wq/wk)
    wv_d = nc.declare_dram_parameter("wv", [128, KT * E], FP16, isOutput=False)
    # wo[p, j*1024 + o] = Wo[j*128+p, o]
    wo_d = nc.declare_dram_parameter("wo", [128, 2 * OUT], FP16, isOutput=False)
    bq_d = nc.declare_dram_parameter("bq", [128, E // 128], FP32, isOutput=False)
    bk_d = nc.declare_dram_parameter("bk", [128, E // 128], FP32, isOutput=False)
    bv_d = nc.declare_dram_parameter("bv", [128, E // 128], FP32, isOutput=False)
    id_d = nc.declare_dram_parameter("ident", [128, 128], FP16, isOutput=False)
    bo_d = nc.declare_dram_parameter("bo4", [128, OUT // 128], FP32, isOutput=False)
    outT = nc.declare_dram_parameter("outT", [OUT, S], FP32, isOutput=True)

    with tile.TileContext(nc) as tc:
        with (
            tc.tile_pool(name="w", bufs=1) as wpool,
            tc.tile_pool(name="work", bufs=3) as work,
            tc.tile_pool(name="cnp", bufs=1) as cnpool,
            tc.tile_pool(name="ps", bufs=2, space="PSUM") as psp,
            tc.tile_pool(name="ctxps", bufs=3, space="PSUM") as ctxp,
            tc.tile_pool(name="pop", bufs=1, space="PSUM") as pop,
        ):
            # ---- persistent SBUF residents ----
            xall = wpool.tile([128, 8 * S], FP16, tag="xall")
            wq_s = wpool.tile([128, KT * E], FP16, tag="wq")
            wk_s = wpool.tile([128, KT * E], FP16, tag="wk")
            wv_s = wpool.tile([128, KT * E], FP16, tag="wv")
            wo_s = wpool.tile([128, 2 * OUT], FP16, tag="wo")
            bq_s = wpool.tile([128, E // 128], FP32, tag="bq")
            bk_s = wpool.tile([128, E // 128], FP32, tag="bk")
            bv_s = wpool.tile([128, E // 128], FP32, tag="bv")
            ident = wpool.tile([128, 128], FP16, tag="ident")
            bo_s = wpool.tile([128, OUT // 128], FP32, tag="bo")
            ones_f = wpool.tile([1, 64], FP16, tag="ones_f")
            ones_v = wpool.tile([128, EV], FP32, tag="ones_v")
            # bo/4 broadcast to 256 columns per mo-group for the one-shot
            # epilogue bias adds: bo_cols[p, mo*256 + c] = bo[mo*128+p]/4
            bo_cols = wpool.tile([128, 2048], FP32, tag="bo_cols")
            vts16 = [wpool.tile([128, S], FP16, tag=f"vT{m}", name=f"vT{m}") for m in range(2)]
            qts = [wpool.tile([128, S], FP16, tag=f"qt{m}", name=f"qt{m}") for m in range(2)]
            kts = [wpool.tile([128, S], FP16, tag=f"kt{m}", name=f"kt{m}") for m in range(2)]
            vts = [wpool.tile([128, EV], FP16, tag=f"vt{m}", name=f"vt{m}") for m in range(MT)]
            cns = [cnpool.tile([128, S], FP16, tag=f"cn{m}", name=f"cn{m}") for m in range(2)]

            # ---- input DMAs: contiguous, need-ordered, split across the
            # Sync and GpSimd descriptor queues. Sync carries the
            # first-exp critical path (wq, x n=0, wk); GpSimd the v path
            # and later x chunks ----
            C = 2 * S  # one n-chunk of the x image
            nc.sync.dma_start(out=wq_s[:], in_=wq_d[:, :])
            nc.sync.dma_start(out=xall[:, 0:C], in_=xall_d[:, 0:C])
            nc.sync.dma_start(out=wv_s[:], in_=wv_d[:, :])
            nc.sync.dma_start(out=wk_s[:], in_=wk_d[:, :])
            nc.sync.dma_start(out=bq_s[:], in_=bq_d[:, :])
            nc.sync.dma_start(out=bk_s[:], in_=bk_d[:, :])
            nc.sync.dma_start(out=bv_s[:], in_=bv_d[:, :])
            nc.sync.dma_start(out=ident[:], in_=id_d[:, :])
            nc.sync.dma_start(out=xall[:, C : 2 * C], in_=xall_d[:, C : 2 * C])
            nc.sync.dma_start(out=xall[:, 2 * C : 3 * C], in_=xall_d[:, 2 * C : 3 * C])
            nc.sync.dma_start(out=xall[:, 3 * C : 4 * C], in_=xall_d[:, 3 * C : 4 * C])
            nc.sync.dma_start(out=wo_s[:], in_=wo_d[:, :])
            nc.sync.dma_start(out=bo_s[:], in_=bo_d[:, :])
            nc.vector.memset(ones_f[:], 1.0)
            nc.vector.memset(ones_v[:], 1.0)
            nc.vector.memset(bo_cols[:], 0.0)
            for mo in range(8):
                nc.vector.tensor_scalar_add(
                    bo_cols[:, mo * 256 : (mo + 1) * 256],
                    bo_cols[:, mo * 256 : (mo + 1) * 256],
                    bo_s[:, mo : mo + 1],
                )
            # the softmax-denominator ones columns of v_ext are constant:
            # write them once; the transpose evictions fill the other columns
            for m in range(MT):
                nc.vector.memset(
                    vts[m][:].rearrange("p (h e) -> p h e", e=65)[:, :, 64:65], 1.0
                )

            def xs(n, k):
                """x image chunk [128, 512] for (token-slice n, d-tile k)."""
                off = (n * KT + k) * 512
                return xall[:, off : off + 512]

            # Warm the PE clock (HAM un-throttles after ~3.4us sustained)
            # while the first DMAs stream in.
            wu = wpool.tile([128, 512], FP16, tag="wu")
            nc.vector.memset(wu[:], 0.0)
            wups = psp.tile([128, 1024], FP32, tag="S", name="wups")
            for i in range(14):
                nc.tensor.matmul(
                    wups[:, 0:512], lhsT=wu[:, 0:128], rhs=wu[:], start=True, stop=True
                )

            def qk_group(hp, which, n, on_pop=False):
                w_s, dst, bias = {
                    "q": (wq_s, qts, bq_s),
                    "k": (wk_s, kts, bk_s),
                    "v": (wv_s, vts16, bv_s),
                }[which]
                # projections in the pop bank don't couple the psp (scores)
                # slot rotation to their DMA/eviction chain
                if which == "v" or on_pop:
                    ps = pop.tile([128, 512], FP32, tag="po", name="psv")
                else:
                    ps = psp.tile([128, 512], FP32, tag="S", name="ps")
                for k in range(KT):
                    nc.tensor.matmul(
                        ps[:],
                        lhsT=w_s[:, k * E + hp * 128 : k * E + hp * 128 + 128],
                        rhs=xs(n, k),
                        start=(k == 0),
                        stop=(k == KT - 1),
                    )
                nc.vector.tensor_add(
                    dst[hp][:, n * 512 : (n + 1) * 512],
                    ps[:],
                    bias[:, hp : hp + 1].to_broadcast((128, 512)),
                )

            def v_transpose(m):
                """vts16 [e,t] block -> vts[m] [t, 4x(64|one)] via PE
                transpose (identity matmul); ones columns are pre-memset."""
                for ei in range(2):
                    tp = ctxp.tile([128, 128], FP16, tag="ctx", name="tp")
                    nc.tensor.transpose(
                        tp[:], vts16[ei][:, m * 128 : (m + 1) * 128], ident[:]
                    )
                    nc.vector.tensor_mul(
                        vts[m][:].rearrange("p (h e) -> p h e", e=65)[
                            :, 2 * ei : 2 * ei + 2, 0:64
                        ],
                        tp[:].rearrange("p (h e) -> p h e", h=2),
                        ones_v[:].rearrange("p (h e) -> p h e", e=65)[:, 0:2, 0:64],
                    )

            def normalize_p1_pair(ctx_a, ctx_b, qw):
                cs = work.tile([65, 1024], FP32, tag="cs", bufs=5, name="cs_ab")
                nc.vector.tensor_copy(cs[:, 0:qw], ctx_a[0:65, 0:qw])
                nc.vector.tensor_copy(cs[:, qw : 2 * qw], ctx_b[0:65, 0:qw])
                ln = work.tile([1, 1024], FP32, tag="lns", bufs=4, name="ln_ab")
                nc.scalar.activation(
                    ln[:, 0 : 2 * qw], cs[64:65, 0 : 2 * qw],
                    mybir.ActivationFunctionType.Ln,
                )
                r = work.tile([1, 1024], FP16, tag="recip", bufs=4, name="r_ab")
                nc.scalar.activation(
                    r[:, 0 : 2 * qw], ln[:, 0 : 2 * qw],
                    mybir.ActivationFunctionType.Exp, scale=-1.0,
                )
                return cs, r

            def normalize_p2(cs, r, hp, a, q0, qw):
                bc = ctxp.tile([65, 512], FP32, tag="ctx")
                nc.tensor.matmul(
                    bc[0:64, 0:qw],
                    lhsT=ones_f[:],
                    rhs=r[0:1, qw * a : qw * a + qw],
                    start=True,
                    stop=True,
                )
                nc.vector.tensor_mul(
                    cns[hp][64 * a : 64 * a + 64, q0 : q0 + qw],
                    cs[0:64, qw * a : qw * a + qw],
                    bc[0:64, 0:qw],
                )

            def out_proj_group(q0, qw, mo, pool=None):
                if pool is None:
                    pool, ptag = pop, "po"
                elif pool is psp:
                    ptag = "S"
                else:
                    ptag = "ctx"
                ps = pool.tile([128, 512], FP32, tag=ptag, name="ps_o")
                for k in range(2):
                    nc.tensor.matmul(
                        ps[:, 0:qw],
                        lhsT=wo_s[:, k * OUT + mo * 128 : k * OUT + mo * 128 + 128],
                        rhs=cns[k][:, q0 : q0 + qw],
                        start=(k == 0),
                        stop=(k == 1),
                    )
                ot = work.tile([128, 512], FP32, tag="ot", bufs=8)
                nc.vector.tensor_scalar_add(ot[:, 0:qw], ps[:, 0:qw], bo_s[:, mo : mo + 1])
                q_eng = nc.sync if mo % 2 == 0 else nc.gpsimd
                q_eng.dma_start(
                    out=outT[mo * 128 : (mo + 1) * 128, q0 : q0 + qw],
                    in_=ot[:, 0:qw],
                )

            def attn_slice(hp, q0, qw, fillers, last=False):
                ctx_a = ctxp.tile([65, 512], FP32, tag="ctx", name="ctx_a")
                ctx_b = ctxp.tile([65, 512], FP32, tag="ctx", name="ctx_b")
                hb = 512  # head-b scores go in the second PSUM bank
                for m in range(MT):
                    for f in fillers.get(m, ()):
                        with tc.high_priority(offset=-200):
                            f()
                    sps = psp.tile([128, 1024], FP32, tag="S", name="sps")
                    nc.tensor.matmul(
                        sps[:, 0:qw],
                        lhsT=kts[hp][0:64, m * 128 : (m + 1) * 128],
                        rhs=qts[hp][0:64, q0 : q0 + qw],
                        start=True,
                        stop=True,
                    )
                    nc.tensor.matmul(
                        sps[:, hb : hb + qw],
                        lhsT=kts[hp][64:128, m * 128 : (m + 1) * 128],
                        rhs=qts[hp][64:128, q0 : q0 + qw],
                        start=True,
                        stop=True,
                    )
                    ee = work.tile([128, 1024], FP16, tag="E", bufs=5)
                    if qw == 512:
                        nc.scalar.activation(
                            ee[:], sps[:], mybir.ActivationFunctionType.Exp, scale=SCALE
                        )
                    else:
                        s3 = sps[:].rearrange("p (h c) -> p h c", h=2)[:, :, 0:qw]
                        e3 = ee[:].rearrange("p (h c) -> p h c", h=2)[:, :, 0:qw]
                        nc.scalar.activation(
                            e3, s3, mybir.ActivationFunctionType.Exp, scale=SCALE
                        )
                    ha = 2 * hp
                    ctx2 = tc.high_priority(offset=-40)
                    ctx2.__enter__()
                    nc.tensor.matmul(
                        ctx_a[0:65, 0:qw],
                        lhsT=vts[m][:, ha * 65 : ha * 65 + 65],
                        rhs=ee[:, 0:qw],
                        start=(m == 0),
                        stop=(m == MT - 1),
                    )
                    nc.tensor.matmul(
                        ctx_b[0:65, 0:qw],
                        lhsT=vts[m][:, (ha + 1) * 65 : (ha + 1) * 65 + 65],
                        rhs=ee[:, hb : hb + qw],
                        start=(m == 0),
                        stop=(m == MT - 1),
                    )
                    ctx2.__exit__(None, None, None)
                if last:
                    return ctx_a, ctx_b
                cs, r = normalize_p1_pair(ctx_a, ctx_b, qw)
                return [(cs, r, hp, 0, q0, qw), (cs, r, hp, 1, q0, qw)]

            # ---- emission schedule (software pipeline) ----
            qk_group(0, "q", 0)
            qk_group(0, "k", 0)
            with tc.high_priority(offset=-200):
                qk_group(0, "v", 0)
                qk_group(1, "v", 0)
                v_transpose(0)
                v_transpose(1)

            def tr(m):
                return lambda: v_transpose(m)

            def vg(ei, n):
                return lambda: qk_group(ei, "v", n)

            # attn0 nq=0: vT projections + per-m transposes just-in-time
            fill0 = {
                1: [tr(2)],
                2: [tr(3), vg(0, 1)],
                3: [vg(1, 1)],
                4: [lambda: qk_group(0, "k", 1), tr(4)],
                5: [tr(5), tr(6)],
                6: [vg(0, 2), tr(7)],
                7: [vg(1, 2)],
                8: [lambda: qk_group(0, "k", 2), tr(8)],
                9: [tr(9), tr(10)],
                10: [vg(0, 3), tr(11)],
                11: [vg(1, 3)],
                12: [lambda: qk_group(0, "k", 3), tr(12)],
                13: [tr(13), lambda: qk_group(0, "q", 1)],
                14: [tr(14)],
                15: [tr(15)],
            }
            pending = attn_slice(0, 0, 512, fill0)

            def norm_fillers(pending, at=(7, 9)):
                return {
                    s: [lambda p=p: normalize_p2(*p)]
                    for s, p in zip(at, pending)
                }

            def merge(f1, f2):
                out = dict(f1)
                for k, v in f2.items():
                    out[k] = out.get(k, []) + v
                return out

            qk1 = [("q", 0), ("k", 0), ("k", 1), ("k", 2), ("k", 3), ("q", 1), ("q", 2), ("q", 3)]

            def qg(hp, which, n):
                return lambda: qk_group(hp, which, n, on_pop=True)

            fills = {
                1: {3: [qg(1, *qk1[0])], 7: [qg(1, *qk1[1])],
                    12: [qg(0, "q", 2)]},
                2: {3: [qg(1, *qk1[2])], 7: [qg(1, *qk1[3])],
                    12: [qg(0, "q", 3)]},
                3: {3: [qg(1, *qk1[4])], 6: [qg(1, *qk1[5])],
                    10: [qg(1, *qk1[6])], 13: [qg(1, *qk1[7])]},
            }
            for nq in range(1, NQ):
                pending = attn_slice(0, nq * 512, 512, merge(fills[nq], norm_fillers(pending)))

            # attn1 nq=0..2 (baseline filler placement)
            for nq in range(NQ - 1):
                fill = norm_fillers(pending)
                if nq > 0:
                    op = {}
                    for mo in range(OUT // 128):
                        s = 10 + mo if mo < 6 else (14 if mo == 6 else 15)
                        op.setdefault(s, []).append(
                            lambda n=nq - 1, mo=mo: out_proj_group(n * 512, 512, mo)
                        )
                    fill = merge(fill, op)
                pending = attn_slice(1, nq * 512, 512, fill)

            # half-slice A of the last query slice: norm(1,2) at (5,7),
            # out-proj of nq=2 (mo 0-3) at even late steps
            fill = norm_fillers(pending, at=(5, 7))
            op = {}
            for mo in range(4):
                op.setdefault(8 + 2 * mo, []).append(
                    lambda mo=mo: out_proj_group(2 * 512, 512, mo)
                )
            pending = attn_slice(1, 1536, 256, merge(fill, op))

            # half-slice B: rest of out-proj(nq=2), norm(A) at (4,6), then
            # out-proj(A) at steps 7..14
            fill = norm_fillers(pending, at=(4, 6))
            op = {}
            for mo in range(4):
                op.setdefault(mo, []).append(
                    lambda mo=mo: out_proj_group(2 * 512, 512, 4 + mo)
                )
            for mo in range(OUT // 128):
                op.setdefault(7 + mo, []).append(
                    lambda mo=mo: out_proj_group(1536, 256, mo)
                )
            pending = attn_slice(1, 1792, 256, merge(fill, op))
            # Final-slice epilogue, latency-trimmed: 4 output groups packed
            # per PSUM tile at 256-column offsets, all 8 matmul pairs
            # emitted back-to-back, then ONE wide fp32 bias add per tile
            # (bias pre-broadcast into bo_cols) and 4 DMAs each - no
            # per-group PE<->DVE ping-pong pacing the drain.
            qw = 256
            for p in pending:
                normalize_p2(*p)
            tails = []
            for half in range(2):
                ps = psp.tile([128, 1024], FP32, tag="S", name=f"tail{half}")
                for g in range(4):
                    mo = half * 4 + g
                    for k in range(2):
                        nc.tensor.matmul(
                            ps[:, g * 256 : g * 256 + qw],
                            lhsT=wo_s[:, k * OUT + mo * 128 : k * OUT + mo * 128 + 128],
                            rhs=cns[k][:, 1792:2048],
                            start=(k == 0),
                            stop=(k == 1),
                        )
                tails.append(ps)
            for half, ps in enumerate(tails):
                ot = work.tile([128, 1024], FP32, tag="ot32", bufs=2, name=f"ot32_{half}")
                nc.vector.tensor_add(
                    ot[:], ps[:], bo_cols[:, half * 1024 : (half + 1) * 1024]
                )
                for g in range(4):
                    mo = half * 4 + g
                    q_eng = nc.sync if mo % 2 == 0 else nc.gpsimd
                    q_eng.dma_start(
                        out=outT[mo * 128 : (mo + 1) * 128, 1792:2048],
                        in_=ot[:, g * 256 : g * 256 + qw],
                    )

    _prune_waits(nc)
    _split_waits(nc)
    return nc


_PROGRAM = None


def _get_program():
    global _PROGRAM
    if _PROGRAM is None:
        _PROGRAM = build_program()
    return _PROGRAM


def _shard_inputs(x, Wq, bq, Wk, bk, Wv, bv, Wo, bo):
    f16 = np.float16
    in_maps = []
    for c in range(NCORES):
        b = c // 4
        g = c % 4
        hs = slice(g * HPC, (g + 1) * HPC)

        # x image: [p, (n, k, c)] = x[b, n*512+c, k*128+p]
        xr = x[b].reshape(4, 512, KT, 128)  # n, c, k, p
        xi = np.ascontiguousarray(xr.transpose(3, 0, 2, 1)).reshape(128, 8 * S)

        def wimg(W):  # [p, (k, e)] = W[k*128+p, e]
            wf = W[hs].transpose(1, 0, 2).reshape(D, E)
            return np.ascontiguousarray(
                wf.reshape(KT, 128, E).transpose(1, 0, 2)
            ).reshape(128, KT * E)


        wof = Wo[g * E : (g + 1) * E, :]
        woi = np.ascontiguousarray(
            wof.reshape(2, 128, OUT).transpose(1, 0, 2)
        ).reshape(128, 2 * OUT)


        in_maps.append(
            {
                "xall": xi.astype(f16),
                "wq": wimg(Wq).astype(f16),
                "wk": wimg(Wk).astype(f16),
                "wv": wimg(Wv).astype(f16),
                "wo": woi.astype(f16),
                "bq": np.ascontiguousarray(bq[hs].reshape(E // 128, 128).T).astype(np.float32),
                "bk": np.ascontiguousarray(bk[hs].reshape(E // 128, 128).T).astype(np.float32),
                "bv": np.ascontiguousarray(bv[hs].reshape(E // 128, 128).T).astype(np.float32),
                "ident": np.eye(128, dtype=np.float16),
                "bo4": np.ascontiguousarray(
                    (bo.astype(np.float32) * 0.25).reshape(OUT // 128, 128).T
                ).astype(np.float32),
            }
        )
    return in_maps


def kernel(x, Wq, bq, Wk, bk, Wv, bv, Wo, bo, _trace=False, _result_box=None):
    in_maps = _shard_inputs(
        np.asarray(x, np.float32),
        np.asarray(Wq, np.float32),
        np.asarray(bq, np.float32),
        np.asarray(Wk, np.float32),
        np.asarray(bk, np.float32),
        np.asarray(Wv, np.float32),
        np.asarray(bv, np.float32),
        np.asarray(Wo, np.float32),
        np.asarray(bo, np.float32),
    )
    nc = _get_program()
    res = run_bass_kernel_spmd(nc, in_maps, list(range(NCORES)), trace=_trace)
    if _result_box is not None:
        _result_box.append(res)

    out = np.empty((B, S, OUT), dtype=np.float32)
    for b in range(B):
        acc = res.results[4 * b]["outT"].astype(np.float32).copy()
        for g in range(1, 4):
            acc += res.results[4 * b + g]["outT"]
        out[b] = acc.T
    return out



# revision 48
# speedup vs baseline: 1.2336x; 1.2336x over previous
"""Multi-head attention (B=2, S=2048, D=1024, H=16, dh=64) on 8 Trainium2 cores.

Sharding: head-tensor-parallel x batch. Core c owns batch b=c//4 and heads
4*(c%4)..4*(c%4)+3 (256 of the 1024 ctx dims). Each core computes its heads'
Q/K/V projections, attention, and a partial output projection against its
256 rows of Wo (+ bo/4 so the 4 partials per batch sum to one bo). The host
unshard step sums the 4 partial outputs per batch.

Per-core kernel (fp16 matmul operands, fp32 PSUM accumulation) — v3:
  Same math as the fp16 baseline: qT/kT = W.T@x.T computed in transposed
  form; head pairs row-packed so the two K=64 score matmuls of a pair run
  concurrently in PE row groups; exp on ScalarE ([128,1024] per key tile,
  ~1.1us — the ACT engine is the # 2 bottleneck at ~160us); A@V with a
  stationary [V | ones] operand so softmax denominators fall out of the
  accumulation; 1/rowsum via Ln+Exp(-x) on ACT; per-head broadcast via K=1
  fp16 matmul; out^T = Wo.T @ ctx_norm^T.

  v3 scheduling changes vs the 259.8us baseline (trace-driven):
  - Inputs are host-prepacked into SBUF-image layouts so each tensor is ONE
    contiguous DMA descriptor (12 total vs ~46).
  - Output DMAs alternate Sync/GpSimd queues.
  - (v3 ran the last query slice as two 256-wide half-slices; superseded
    by the v5 full-width last slice + packed epilogue.)

  v4 changes (trace-driven; measured engine truths: ACT paces the body at
  ~1.0us per [128,1024] exp with Ln/Exp(-1) on its inter-slice chain; PE
  TensorMatrix busy ~191us with K=64 score pairs genuinely concurrent in
  row quadrants; ~6.5us fixed NEFF preamble and ~5us teardown after the
  last output DMA):
  - _prune_waits: drops semaphore waits implied by same-engine program
    order or dominated by an earlier wait (1026 -> 708 waited
    instructions, 372 -> 90 split-wait NOPs).
  - Final-slice epilogue: the 8 output-projection groups pack 4 per PSUM
    tile at 256-column offsets, all matmul pairs emitted back-to-back,
    then ONE wide fp32 bias add per tile against a pre-broadcast bo_cols
    image, 4 DMAs each across both queues - removes the per-group
    PE<->DVE ping-pong that paced the drain at ~1.05us/group.
  - Filler priority offset -200 -> -20: -200 hoisted a slice's whole
    filler budget to its entry (5.5us ACT gaps at attn0 slice
    boundaries); -20 lets fillers spread into per-step PE slack.
    (Differentiated priorities -200/-60/-20 measured worse; uniform -20
    is the local optimum.)
  - Mid-kernel output DMAs issue on the GpSimd queue (the Sync sequencer
    measured 108% oversubscribed, delaying the final DMAs + teardown);
    the two epilogue DMAs are ONE strided descriptor per PSUM-tile half
    covering 4 mo-blocks (outT viewed [p, m, s]), killing the serial
    ~640ns/DMA descriptor-gen tail.
  - Slice-0 k-projection fillers sit two m-slots before their consumers;
    slice-0 exp gaps that remain (~3us at m=4/8/12) track the 1MB x-chunk
    DMA arrivals - an HBM-bandwidth floor, not a scheduling artifact.
  v5: the last query slice runs FULL-width (512) instead of two 256-wide
  half-slices, cutting ~3.6us of ACT busy (16 exps + 1 norm pair instead
  of 32 + 2); its out-projection is a 6-tile packed epilogue (2 psp tiles
  holding 2 groups each with wide bias adds + strided 2-block DMAs, plus
  3 ctxp / 1 pop tiles with per-group adds). 216.6us, twice reproduced to
  within 3ns. The earlier NaN on this structure was _prune_waits'
  dominated-wait rule (rule 2) - unsound here; the pass is now disabled
  entirely (same-engine rule alone measured net-neutral).
  Rejected by measurement: DVE reciprocal (5.8us/slice - iterative op),
  tensor_tensor/tensor_scalar divide or pow (no such ISA op on DVE/Pool),
  custom-DVE reciprocal_approx_fast (walrus "ISA wrong length"), fp8
  DoubleRow q/k projections (host-simulated max-rel 3.6e-2 > 2e-2 gate),
  input-DMA queue splitting (halves critical-path DMA bandwidth), x-chunk
  interleaved projection prologue (HAM stays cold on DMA-paced gaps).
"""

import numpy as np

import bass_rust
import concourse.bass as bass
import concourse.mybir as mybir
import concourse.tile as tile
from concourse.bass_utils import run_bass_kernel_spmd

B = 2
S = 2048
D = 1024
H = 16
DH = 64
OUT = 1024
NCORES = 8
HPC = H // 4  # heads per core = 4
E = HPC * DH  # 256 ctx dims per core
EV = HPC * (DH + 1)  # 260: v with interleaved ones columns

FP16 = mybir.dt.float16
FP32 = mybir.dt.float32

SCALE = 1.0 / float(np.sqrt(DH))

KT = D // 128  # 8 k-tiles for projections
MT = S // 128  # 16 key-token tiles
NQ = S // 512  # 4 query slices of 512


def _split_waits(nc, maxw=1):
    """This container's walrus rejects instructions carrying more than one
    semaphore wait ("Too many sync wait commands"); hoist extras onto
    standalone same-engine nops, preserving per-engine program order."""
    for bb in nc.main_func.blocks:
        new_il = []
        for inst in bb.instructions:
            si = inst.sync_info
            if si is not None and si.on_wait and len(si.on_wait) > maxw:
                waits = list(si.on_wait)
                for j, w in enumerate(waits[:-maxw]):
                    nop = mybir.InstNoOp(
                        name=f"{inst.name}-ws{j}", ins=[], outs=[], engine=inst.engine
                    )
                    nop.sync_info = bass_rust.SyncInfo(on_wait=[w], on_update=[])
                    new_il.append(nop)
                inst.sync_info = bass_rust.SyncInfo(
                    on_wait=waits[-maxw:], on_update=list(si.on_update)
                )
            new_il.append(inst)
        bb.instructions = new_il


def _prune_waits(nc):
    """Drop provably-satisfied semaphore waits before _split_waits:

    1. Same-engine waits: engines execute/dispatch in program order, and
       every sem-inc updater of an engine-local semaphore fires at
       instruction completion in that order. A wait on a semaphore whose
       only updaters are earlier instructions of the SAME engine, with
       threshold <= the number of those updates already emitted, is
       implied by program order. (DMA-queue sems are excluded: their
       increments fire at asynchronous DMA completion.)
    2. Dominated waits: semaphores are monotone counters, and an engine
       dispatches in order, so a wait with threshold <= the max threshold
       this engine has already waited for on the same semaphore is a
       no-op.
    Both classes otherwise cost sequencer wait-checks and, via
    _split_waits, standalone NOP dispatches on the busiest engines."""
    # sem id -> set of updater engines; and async (DMA) sems
    updaters = {}
    async_sems = set()
    for bb in nc.main_func.blocks:
        for inst in bb.instructions:
            si = inst.sync_info
            if si is None:
                continue
            for u in si.on_update:
                if getattr(u, "update_mode", None) != "sem-inc":
                    async_sems.add(u.id)
                    continue
                updaters.setdefault(u.id, set()).add(inst.engine)
                if type(inst).__name__ in ("InstDMACopy", "InstDMACopyTranspose"):
                    async_sems.add(u.id)

    emitted = {}  # (engine, sem id) -> inc count emitted so far
    waited = {}  # (engine, sem id) -> max threshold already waited
    pruned = [0, 0]
    for bb in nc.main_func.blocks:
        for inst in bb.instructions:
            si = inst.sync_info
            if si is None:
                continue
            eng = inst.engine
            if si.on_wait:
                keep = []
                for w in si.on_wait:
                    if getattr(w, "wait_mode", None) != "sem-ge-imm":
                        keep.append(w)
                        continue
                    sid, val = w.id, w.wait_value
                    if sid not in async_sems:
                        if updaters.get(sid) == {eng} and val <= emitted.get(
                            (eng, sid), 0
                        ):
                            pruned[0] += 1
                            continue
                        # dominated-wait rule disabled: unsound with the
                        # packed epilogue (NaN); same-engine rule alone is
                        # order-safe
                    waited[(eng, sid)] = max(waited.get((eng, sid), -1), val)
                    keep.append(w)
                if len(keep) != len(si.on_wait):
                    inst.sync_info = bass_rust.SyncInfo(
                        on_wait=keep, on_update=list(si.on_update)
                    )
            for u in si.on_update:
                if getattr(u, "update_mode", None) == "sem-inc":
                    k = (eng, u.id)
                    emitted[k] = emitted.get(k, 0) + u.update_value


def build_program():
    nc = bass.Bass()

    # Host-prepacked SBUF images: one contiguous DMA descriptor per tensor.
    # xall[p, (n*8 + k)*512 + c] = x[n*512+c, k*128+p]
    xall_d = nc.declare_dram_parameter("xall", [128, 8 * S], FP16, isOutput=False)
    # wq/wk[p, k*256 + e] = W[k*128+p, e]
    wq_d = nc.declare_dram_parameter("wq", [128, KT * E], FP16, isOutput=False)
    wk_d = nc.declare_dram_parameter("wk", [128, KT * E], FP16, isOutput=False)
    # wv[p, k*256 + e] = Wv[k*128+p, e] (same image as wq/wk)
    wv_d = nc.declare_dram_parameter("wv", [128, KT * E], FP16, isOutput=False)
    # wo[p, j*1024 + o] = Wo[j*128+p, o]
    wo_d = nc.declare_dram_parameter("wo", [128, 2 * OUT], FP16, isOutput=False)
    bq_d = nc.declare_dram_parameter("bq", [128, E // 128], FP32, isOutput=False)
    bk_d = nc.declare_dram_parameter("bk", [128, E // 128], FP32, isOutput=False)
    bv_d = nc.declare_dram_parameter("bv", [128, E // 128], FP32, isOutput=False)
    id_d = nc.declare_dram_parameter("ident", [128, 128], FP16, isOutput=False)
    bo_d = nc.declare_dram_parameter("bo4", [128, OUT // 128], FP32, isOutput=False)
    outT = nc.declare_dram_parameter("outT", [OUT, S], FP32, isOutput=True)

    with tile.TileContext(nc) as tc:
        with (
            tc.tile_pool(name="w", bufs=1) as wpool,
            tc.tile_pool(name="work", bufs=3) as work,
            tc.tile_pool(name="cnp", bufs=1) as cnpool,
            tc.tile_pool(name="ps", bufs=2, space="PSUM") as psp,
            tc.tile_pool(name="ctxps", bufs=3, space="PSUM") as ctxp,
            tc.tile_pool(name="pop", bufs=1, space="PSUM") as pop,
        ):
            # ---- persistent SBUF residents ----
            xall = wpool.tile([128, 8 * S], FP16, tag="xall")
            wq_s = wpool.tile([128, KT * E], FP16, tag="wq")
            wk_s = wpool.tile([128, KT * E], FP16, tag="wk")
            wv_s = wpool.tile([128, KT * E], FP16, tag="wv")
            wo_s = wpool.tile([128, 2 * OUT], FP16, tag="wo")
            bq_s = wpool.tile([128, E // 128], FP32, tag="bq")
            bk_s = wpool.tile([128, E // 128], FP32, tag="bk")
            bv_s = wpool.tile([128, E // 128], FP32, tag="bv")
            ident = wpool.tile([128, 128], FP16, tag="ident")
            bo_s = wpool.tile([128, OUT // 128], FP32, tag="bo")
            ones_f = wpool.tile([1, 64], FP16, tag="ones_f")
            ones_v = wpool.tile([128, EV], FP32, tag="ones_v")
            # bo/4 broadcast to 512 columns per mo-group (mo 0-3) for the
            # one-shot epilogue bias adds: bo_cols[p, mo*512+c] = bo[mo*128+p]/4
            bo_cols = wpool.tile([128, 2048], FP32, tag="bo_cols")
            vts16 = [wpool.tile([128, S], FP16, tag=f"vT{m}", name=f"vT{m}") for m in range(2)]
            qts = [wpool.tile([128, S], FP16, tag=f"qt{m}", name=f"qt{m}") for m in range(2)]
            kts = [wpool.tile([128, S], FP16, tag=f"kt{m}", name=f"kt{m}") for m in range(2)]
            vts = [wpool.tile([128, EV], FP16, tag=f"vt{m}", name=f"vt{m}") for m in range(MT)]
            cns = [cnpool.tile([128, S], FP16, tag=f"cn{m}", name=f"cn{m}") for m in range(2)]

            # ---- input DMAs: contiguous, need-ordered, split across the
            # Sync and GpSimd descriptor queues. Sync carries the
            # first-exp critical path (wq, x n=0, wk); GpSimd the v path
            # and later x chunks ----
            C = 2 * S  # one n-chunk of the x image
            nc.sync.dma_start(out=wq_s[:], in_=wq_d[:, :])
            nc.sync.dma_start(out=xall[:, 0:C], in_=xall_d[:, 0:C])
            nc.sync.dma_start(out=wv_s[:], in_=wv_d[:, :])
            nc.sync.dma_start(out=wk_s[:], in_=wk_d[:, :])
            nc.sync.dma_start(out=bq_s[:], in_=bq_d[:, :])
            nc.sync.dma_start(out=bk_s[:], in_=bk_d[:, :])
            nc.sync.dma_start(out=bv_s[:], in_=bv_d[:, :])
            nc.sync.dma_start(out=ident[:], in_=id_d[:, :])
            nc.sync.dma_start(out=xall[:, C : 2 * C], in_=xall_d[:, C : 2 * C])
            nc.sync.dma_start(out=xall[:, 2 * C : 3 * C], in_=xall_d[:, 2 * C : 3 * C])
            nc.sync.dma_start(out=xall[:, 3 * C : 4 * C], in_=xall_d[:, 3 * C : 4 * C])
            nc.sync.dma_start(out=wo_s[:], in_=wo_d[:, :])
            nc.sync.dma_start(out=bo_s[:], in_=bo_d[:, :])
            nc.vector.memset(ones_f[:], 1.0)
            nc.vector.memset(ones_v[:], 1.0)
            nc.vector.memset(bo_cols[:], 0.0)
            for mo in range(4):
                nc.vector.tensor_scalar_add(
                    bo_cols[:, mo * 512 : (mo + 1) * 512],
                    bo_cols[:, mo * 512 : (mo + 1) * 512],
                    bo_s[:, mo : mo + 1],
                )
            # the softmax-denominator ones columns of v_ext are constant:
            # write them once; the transpose evictions fill the other columns
            for m in range(MT):
                nc.vector.memset(
                    vts[m][:].rearrange("p (h e) -> p h e", e=65)[:, :, 64:65], 1.0
                )

            def xs(n, k):
                """x image chunk [128, 512] for (token-slice n, d-tile k)."""
                off = (n * KT + k) * 512
                return xall[:, off : off + 512]

            # Warm the PE clock (HAM un-throttles after ~3.4us sustained)
            # while the first DMAs stream in.
            wu = wpool.tile([128, 512], FP16, tag="wu")
            nc.vector.memset(wu[:], 0.0)
            wups = psp.tile([128, 1024], FP32, tag="S", name="wups")
            for i in range(14):
                nc.tensor.matmul(
                    wups[:, 0:512], lhsT=wu[:, 0:128], rhs=wu[:], start=True, stop=True
                )

            def qk_group(hp, which, n, on_pop=False):
                w_s, dst, bias = {
                    "q": (wq_s, qts, bq_s),
                    "k": (wk_s, kts, bk_s),
                    "v": (wv_s, vts16, bv_s),
                }[which]
                # projections in the pop bank don't couple the psp (scores)
                # slot rotation to their DMA/eviction chain
                if which == "v" or on_pop:
                    ps = pop.tile([128, 512], FP32, tag="po", name="psv")
                else:
                    ps = psp.tile([128, 512], FP32, tag="S", name="ps")
                for k in range(KT):
                    nc.tensor.matmul(
                        ps[:],
                        lhsT=w_s[:, k * E + hp * 128 : k * E + hp * 128 + 128],
                        rhs=xs(n, k),
                        start=(k == 0),
                        stop=(k == KT - 1),
                    )
                nc.vector.tensor_add(
                    dst[hp][:, n * 512 : (n + 1) * 512],
                    ps[:],
                    bias[:, hp : hp + 1].to_broadcast((128, 512)),
                )

            def v_transpose(m):
                """vts16 [e,t] block -> vts[m] [t, 4x(64|one)] via PE
                transpose (identity matmul); ones columns are pre-memset."""
                for ei in range(2):
                    tp = ctxp.tile([128, 128], FP16, tag="ctx", name="tp")
                    nc.tensor.transpose(
                        tp[:], vts16[ei][:, m * 128 : (m + 1) * 128], ident[:]
                    )
                    nc.vector.tensor_mul(
                        vts[m][:].rearrange("p (h e) -> p h e", e=65)[
                            :, 2 * ei : 2 * ei + 2, 0:64
                        ],
                        tp[:].rearrange("p (h e) -> p h e", h=2),
                        ones_v[:].rearrange("p (h e) -> p h e", e=65)[:, 0:2, 0:64],
                    )

            def normalize_p1_pair(ctx_a, ctx_b, qw):
                cs = work.tile([65, 1024], FP32, tag="cs", bufs=5, name="cs_ab")
                nc.vector.tensor_copy(cs[:, 0:qw], ctx_a[0:65, 0:qw])
                nc.vector.tensor_copy(cs[:, qw : 2 * qw], ctx_b[0:65, 0:qw])
                ln = work.tile([1, 1024], FP32, tag="lns", bufs=4, name="ln_ab")
                nc.scalar.activation(
                    ln[:, 0 : 2 * qw], cs[64:65, 0 : 2 * qw],
                    mybir.ActivationFunctionType.Ln,
                )
                r = work.tile([1, 1024], FP16, tag="recip", bufs=4, name="r_ab")
                nc.scalar.activation(
                    r[:, 0 : 2 * qw], ln[:, 0 : 2 * qw],
                    mybir.ActivationFunctionType.Exp, scale=-1.0,
                )
                return cs, r

            def normalize_p2(cs, r, hp, a, q0, qw):
                bc = ctxp.tile([65, 512], FP32, tag="ctx")
                nc.tensor.matmul(
                    bc[0:64, 0:qw],
                    lhsT=ones_f[:],
                    rhs=r[0:1, qw * a : qw * a + qw],
                    start=True,
                    stop=True,
                )
                nc.vector.tensor_mul(
                    cns[hp][64 * a : 64 * a + 64, q0 : q0 + qw],
                    cs[0:64, qw * a : qw * a + qw],
                    bc[0:64, 0:qw],
                )

            def out_proj_group(q0, qw, mo, pool=None):
                if pool is None:
                    pool, ptag = pop, "po"
                elif pool is psp:
                    ptag = "S"
                else:
                    ptag = "ctx"
                ps = pool.tile([128, 512], FP32, tag=ptag, name="ps_o")
                for k in range(2):
                    nc.tensor.matmul(
                        ps[:, 0:qw],
                        lhsT=wo_s[:, k * OUT + mo * 128 : k * OUT + mo * 128 + 128],
                        rhs=cns[k][:, q0 : q0 + qw],
                        start=(k == 0),
                        stop=(k == 1),
                    )
                ot = work.tile([128, 512], FP32, tag="ot", bufs=8)
                nc.vector.tensor_scalar_add(ot[:, 0:qw], ps[:, 0:qw], bo_s[:, mo : mo + 1])
                q_eng = nc.gpsimd
                q_eng.dma_start(
                    out=outT[mo * 128 : (mo + 1) * 128, q0 : q0 + qw],
                    in_=ot[:, 0:qw],
                )

            def attn_slice(hp, q0, qw, fillers, last=False):
                ctx_a = ctxp.tile([65, 512], FP32, tag="ctx", name="ctx_a")
                ctx_b = ctxp.tile([65, 512], FP32, tag="ctx", name="ctx_b")
                hb = 512  # head-b scores go in the second PSUM bank
                for m in range(MT):
                    for f in fillers.get(m, ()):
                        f()
                    sps = psp.tile([128, 1024], FP32, tag="S", name="sps")
                    nc.tensor.matmul(
                        sps[:, 0:qw],
                        lhsT=kts[hp][0:64, m * 128 : (m + 1) * 128],
                        rhs=qts[hp][0:64, q0 : q0 + qw],
                        start=True,
                        stop=True,
                    )
                    nc.tensor.matmul(
                        sps[:, hb : hb + qw],
                        lhsT=kts[hp][64:128, m * 128 : (m + 1) * 128],
                        rhs=qts[hp][64:128, q0 : q0 + qw],
                        start=True,
                        stop=True,
                    )
                    ee = work.tile([128, 1024], FP16, tag="E", bufs=5)
                    if qw == 512:
                        nc.scalar.activation(
                            ee[:], sps[:], mybir.ActivationFunctionType.Exp, scale=SCALE
                        )
                    else:
                        s3 = sps[:].rearrange("p (h c) -> p h c", h=2)[:, :, 0:qw]
                        e3 = ee[:].rearrange("p (h c) -> p h c", h=2)[:, :, 0:qw]
                        nc.scalar.activation(
                            e3, s3, mybir.ActivationFunctionType.Exp, scale=SCALE
                        )
                    ha = 2 * hp
                    ctx2 = tc.high_priority(offset=-40)
                    ctx2.__enter__()
                    nc.tensor.matmul(
                        ctx_a[0:65, 0:qw],
                        lhsT=vts[m][:, ha * 65 : ha * 65 + 65],
                        rhs=ee[:, 0:qw],
                        start=(m == 0),
                        stop=(m == MT - 1),
                    )
                    nc.tensor.matmul(
                        ctx_b[0:65, 0:qw],
                        lhsT=vts[m][:, (ha + 1) * 65 : (ha + 1) * 65 + 65],
                        rhs=ee[:, hb : hb + qw],
                        start=(m == 0),
                        stop=(m == MT - 1),
                    )
                    ctx2.__exit__(None, None, None)
                if last:
                    return ctx_a, ctx_b
                cs, r = normalize_p1_pair(ctx_a, ctx_b, qw)
                return [(cs, r, hp, 0, q0, qw), (cs, r, hp, 1, q0, qw)]

            # ---- emission schedule (software pipeline) ----
            qk_group(0, "q", 0)
            qk_group(0, "k", 0)
            with tc.high_priority(offset=-200):
                qk_group(0, "v", 0)
                qk_group(1, "v", 0)
                v_transpose(0)
                v_transpose(1)

            def prio(off, fn):
                def g():
                    with tc.high_priority(offset=off):
                        fn()
                return g

            def tr(m):
                return prio(-20, lambda: v_transpose(m))

            def vg(ei, n):
                return prio(-20, lambda: qk_group(ei, "v", n))

            # attn0 nq=0: vT projections + per-m transposes just-in-time
            fill0 = {
                1: [tr(2)],
                2: [prio(-20, lambda: qk_group(0, "k", 1)), tr(3), vg(0, 1)],
                3: [vg(1, 1)],
                4: [tr(4)],
                5: [tr(5), tr(6)],
                6: [prio(-20, lambda: qk_group(0, "k", 2)), vg(0, 2), tr(7)],
                7: [vg(1, 2)],
                8: [tr(8)],
                9: [tr(9), tr(10)],
                10: [prio(-20, lambda: qk_group(0, "k", 3)), vg(0, 3), tr(11)],
                11: [vg(1, 3)],
                12: [tr(12)],
                13: [tr(13), prio(-20, lambda: qk_group(0, "q", 1))],
                14: [tr(14)],
                15: [tr(15)],
            }
            pending = attn_slice(0, 0, 512, fill0)

            def norm_fillers(pending, at=(7, 9)):
                return {
                    s: [prio(-20, lambda p=p: normalize_p2(*p))]
                    for s, p in zip(at, pending)
                }

            def merge(f1, f2):
                out = dict(f1)
                for k, v in f2.items():
                    out[k] = out.get(k, []) + v
                return out

            qk1 = [("q", 0), ("k", 0), ("k", 1), ("k", 2), ("k", 3), ("q", 1), ("q", 2), ("q", 3)]

            def qg(hp, which, n):
                return prio(-20, lambda: qk_group(hp, which, n, on_pop=True))

            fills = {
                1: {3: [qg(1, *qk1[0])], 7: [qg(1, *qk1[1])],
                    12: [qg(0, "q", 2)]},
                2: {3: [qg(1, *qk1[2])], 7: [qg(1, *qk1[3])],
                    12: [qg(0, "q", 3)]},
                3: {3: [qg(1, *qk1[4])], 6: [qg(1, *qk1[5])],
                    10: [qg(1, *qk1[6])], 13: [qg(1, *qk1[7])]},
            }
            for nq in range(1, NQ):
                pending = attn_slice(0, nq * 512, 512, merge(fills[nq], norm_fillers(pending)))

            # attn1 nq=0..2 (baseline filler placement)
            for nq in range(NQ - 1):
                fill = norm_fillers(pending)
                if nq > 0:
                    op = {}
                    for mo in range(OUT // 128):
                        s = 10 + mo if mo < 6 else (14 if mo == 6 else 15)
                        op.setdefault(s, []).append(
                            prio(-20, lambda n=nq - 1, mo=mo: out_proj_group(n * 512, 512, mo))
                        )
                    fill = merge(fill, op)
                pending = attn_slice(1, nq * 512, 512, fill)

            # Last query slice runs FULL-width (512): the two 256-wide
            # half-slices cost +3.6us of ACT busy (32 small exps + 2 norm
            # pairs vs 16 + 1); the packed/strided epilogue below makes the
            # old slow-drain rationale obsolete. out-proj of nq=2 rides as
            # fillers; the last slice's own out-proj is the epilogue.
            fill = norm_fillers(pending, at=(5, 7))
            op = {}
            for mo in range(OUT // 128):
                op.setdefault(8 + mo if mo < 7 else 15, []).append(
                    prio(-20, lambda mo=mo: out_proj_group(2 * 512, 512, mo))
                )
            pending = attn_slice(1, 1536, 512, merge(fill, op))
            qw = 512
            for p in pending:
                normalize_p2(*p)
            # Epilogue: groups 0-3 pack 2-per-psp-tile (512-col offsets,
            # wide fp32 bias add vs pre-broadcast bo_cols, one strided DMA
            # per tile); groups 4-7 use the now-idle ctxp/pop banks with
            # per-group adds.
            tails = []
            for half in range(2):
                ps = psp.tile([128, 1024], FP32, tag="S", name=f"tail{half}")
                for g in range(2):
                    mo = half * 2 + g
                    for k in range(2):
                        nc.tensor.matmul(
                            ps[:, g * 512 : (g + 1) * 512],
                            lhsT=wo_s[:, k * OUT + mo * 128 : k * OUT + mo * 128 + 128],
                            rhs=cns[k][:, 1536:2048],
                            start=(k == 0),
                            stop=(k == 1),
                        )
                tails.append(ps)
            extra = []
            for j, mo in enumerate(range(4, 8)):
                pool, tag = (ctxp, "ctx") if j < 3 else (pop, "po")
                ps = pool.tile([128, 512], FP32, tag=tag, name=f"tailx{j}")
                for k in range(2):
                    nc.tensor.matmul(
                        ps[:, 0:512],
                        lhsT=wo_s[:, k * OUT + mo * 128 : k * OUT + mo * 128 + 128],
                        rhs=cns[k][:, 1536:2048],
                        start=(k == 0),
                        stop=(k == 1),
                    )
                extra.append((mo, ps))
            for half, ps in enumerate(tails):
                ot = work.tile([128, 1024], FP32, tag="ot32", bufs=2, name=f"ot32_{half}")
                nc.vector.tensor_add(
                    ot[:], ps[:], bo_cols[:, half * 1024 : (half + 1) * 1024]
                )
                q_eng = nc.sync if half == 0 else nc.gpsimd
                q_eng.dma_start(
                    out=outT[:].rearrange("(m p) s -> p m s", p=128)[
                        :, half * 2 : (half + 1) * 2, 1536:2048
                    ],
                    in_=ot[:].rearrange("p (g c) -> p g c", g=2),
                )
            for mo, ps in extra:
                ot = work.tile([128, 512], FP32, tag="ot", bufs=8)
                nc.vector.tensor_scalar_add(
                    ot[:, 0:512], ps[:, 0:512], bo_s[:, mo : mo + 1]
                )
                q_eng = nc.sync if mo % 2 == 0 else nc.gpsimd
                q_eng.dma_start(
                    out=outT[mo * 128 : (mo + 1) * 128, 1536:2048],
                    in_=ot[:, 0:512],
                )

    # _prune_waits(nc) - net-neutral (~0.5us) even with the unsound
    # dominated-wait rule removed; left off for the verified-fast config
    _split_waits(nc)
    return nc


_PROGRAM = None


def _get_program():
    global _PROGRAM
    if _PROGRAM is None:
        _PROGRAM = build_program()
    return _PROGRAM


def _shard_inputs(x, Wq, bq, Wk, bk, Wv, bv, Wo, bo):
    f16 = np.float16
    in_maps = []
    for c in range(NCORES):
        b = c // 4
        g = c % 4
        hs = slice(g * HPC, (g + 1) * HPC)

        # x image: [p, (n, k, c)] = x[b, n*512+c, k*128+p]
        xr = x[b].reshape(4, 512, KT, 128)  # n, c, k, p
        xi = np.ascontiguousarray(xr.transpose(3, 0, 2, 1)).reshape(128, 8 * S)

        def wimg(W):  # [p, (k, e)] = W[k*128+p, e]
            wf = W[hs].transpose(1, 0, 2).reshape(D, E)
            return np.ascontiguousarray(
                wf.reshape(KT, 128, E).transpose(1, 0, 2)
            ).reshape(128, KT * E)


        wof = Wo[g * E : (g + 1) * E, :]
        woi = np.ascontiguousarray(
            wof.reshape(2, 128, OUT).transpose(1, 0, 2)
        ).reshape(128, 2 * OUT)


        in_maps.append(
            {
                "xall": xi.astype(f16),
                "wq": wimg(Wq).astype(f16),
                "wk": wimg(Wk).astype(f16),
                "wv": wimg(Wv).astype(f16),
                "wo": woi.astype(f16),
                "bq": np.ascontiguousarray(bq[hs].reshape(E // 128, 128).T).astype(np.float32),
                "bk": np.ascontiguousarray(bk[hs].reshape(E // 128, 128).T).astype(np.float32),
                "bv": np.ascontiguousarray(bv[hs].reshape(E // 128, 128).T).astype(np.float32),
                "ident": np.eye(128, dtype=np.float16),
                "bo4": np.ascontiguousarray(
                    (bo.astype(np.float32) * 0.25).reshape(OUT // 128, 128).T
                ).astype(np.float32),
            }
        )
    return in_maps


def kernel(x, Wq, bq, Wk, bk, Wv, bv, Wo, bo, _trace=False, _result_box=None):
    in_maps = _shard_inputs(
        np.asarray(x, np.float32),
        np.asarray(Wq, np.float32),
        np.asarray(bq, np.float32),
        np.asarray(Wk, np.float32),
        np.asarray(bk, np.float32),
        np.asarray(Wv, np.float32),
        np.asarray(bv, np.float32),
        np.asarray(Wo, np.float32),
        np.asarray(bo, np.float32),
    )
    nc = _get_program()
    res = run_bass_kernel_spmd(nc, in_maps, list(range(NCORES)), trace=_trace)
    if _result_box is not None:
        _result_box.append(res)

    out = np.empty((B, S, OUT), dtype=np.float32)
    for b in range(B):
        acc = res.results[4 * b]["outT"].astype(np.float32).copy()
        for g in range(1, 4):
            acc += res.results[4 * b + g]["outT"]
        out[b] = acc.T
    return out

